# revision 1
# baseline (speedup 1.0000x reference)
"""Causal self-attention (B=4, T=2048, C=1024, H=16) on 8 Trainium2 cores.

Sharding: tensor-parallel over heads. Core c owns heads (2c, 2c+1):
  - QKV projection: x @ W_attn column-slice (384 cols) -> Q^T, K^T, V^T for
    its 2 heads, computed directly in transposed [dim, token] layout by
    feeding the host-pretransposed x^T as the moving operand.
  - Attention computed in the S^T = [j, i] orientation so the softmax'd
    probabilities are already transposed for the AV matmul (no P transposes).
    The two heads' QK^T matmuls are packed onto the PE array's row groups
    (measured to overlap fully on HW).
  - The softmax denominator rides the AV matmul for free: the stationary
    V operand is augmented with a ones column (head A: [V_A | 1 | 0...],
    head B: [1 | 0... | V_B]) so each AV pass also emits sum_j P[j,i] on a
    partition disjoint from y. A K=1 ones matmul broadcasts the reciprocal
    row across y's partitions for the normalization multiply.
  - Output projection: y^T @ W_proj row-slice -> per-core partial [BT, C];
    partials are summed on the host (row-parallel tensor parallelism).

All three stages are interleaved per batch so DMA (x in / out store), PE
(matmuls), ACT (exp) and DVE (evictions) overlap across stage boundaries.
"""

import numpy as np

N_CORES = 8
B, T, C = 4, 2048, 1024
H, Dh = 16, 64
BT = B * T  # 8192
TCH = 512  # t-chunk (stage 1) and i-chunk (stage 2)

_CACHE = {}


def _build():
    import concourse.bacc as bacc
    import concourse.mybir as mybir
    import concourse.tile as tile
    from concourse.masks import make_identity
    from contextlib import ExitStack

    f32 = mybir.dt.float32
    f32r = mybir.dt.float32r
    bf16 = mybir.dt.bfloat16
    Exp = mybir.ActivationFunctionType.Exp
    add = mybir.AluOpType.add
    mult = mybir.AluOpType.mult
    is_ge = mybir.AluOpType.is_ge

    nc = bacc.Bacc(None, target_bir_lowering=False, debug=False)
    x_t = nc.dram_tensor("x_t", [C, BT], f32r, kind="ExternalInput").ap()
    w_qkv = nc.dram_tensor("w_qkv", [C, 384], f32r, kind="ExternalInput").ap()
    b_qkv = nc.dram_tensor("b_qkv", [128, 3], f32, kind="ExternalInput").ap()
    w_proj = nc.dram_tensor("w_proj", [128, C], f32r, kind="ExternalInput").ap()
    ones_in = nc.dram_tensor("ones_in", [128, 128], f32r, kind="ExternalInput").ap()
    out = nc.dram_tensor("out", [BT, C], f32, kind="ExternalOutput").ap()

    scale = 1.0 / float(np.sqrt(Dh))

    with tile.TileContext(nc) as tc, ExitStack() as ctx:
        persist = ctx.enter_context(tc.tile_pool(name="persist", bufs=1))
        xt_pool = ctx.enter_context(tc.tile_pool(name="xt", bufs=2))
        vs_pool = ctx.enter_context(tc.tile_pool(name="vs", bufs=3))
        p_pool = ctx.enter_context(tc.tile_pool(name="pp", bufs=8))
        rc_pool = ctx.enter_context(tc.tile_pool(name="rc", bufs=3))
        yt_pool = ctx.enter_context(tc.tile_pool(name="yt", bufs=2))
        ob_pool = ctx.enter_context(tc.tile_pool(name="ob", bufs=3))
        # PSUM: 8 banks total: s_ps 2x[128,1024]=4, yA/yB 2x[128,512]=2, work 2
        ps_big = ctx.enter_context(tc.tile_pool(name="psb", bufs=2, space="PSUM"))
        ps_y = ctx.enter_context(tc.tile_pool(name="psy", bufs=2, space="PSUM"))
        ps_work = ctx.enter_context(tc.tile_pool(name="psw", bufs=2, space="PSUM"))

        QT = persist.tile([128, BT], f32r, tag="QT")
        KT = persist.tile([128, BT], f32r, tag="KT")
        # augmented V, bf16: per t-tile gt a [128, 128] stationary block
        #   VnA[:, gt, 0:64] = V_A, [.., 64] = 1.0, rest 0
        #   VnB[:, gt, 0] = 1.0, [.., 64:128] = V_B, rest 0
        VnA = persist.tile([128, 64, 128], bf16, tag="VnA")
        VnB = persist.tile([128, 64, 128], bf16, tag="VnB")
        wq_sb = persist.tile([128, 8, 384], f32r, tag="wq")
        wp_sb = persist.tile([128, C], f32r, tag="wp")
        bq_sb = persist.tile([128, 3], f32, tag="bq")
        ident = persist.tile([128, 128], f32, tag="ident")
        ones1 = persist.tile([128, 128], f32r, tag="ones1")

        # weights via SWDGE ring so they don't queue behind the first x chunk
        nc.gpsimd.dma_start(wq_sb[:], w_qkv.rearrange("(cc p) j -> p cc j", p=128))
        nc.gpsimd.dma_start(wp_sb[:], w_proj[:])
        nc.gpsimd.dma_start(bq_sb[:], b_qkv[:])
        nc.gpsimd.dma_start(ones1[:], ones_in[:])
        make_identity(nc, ident[:])
        nc.gpsimd.memset(VnA[:], 0.0)
        nc.gpsimd.memset(VnB[:], 0.0)
        nc.gpsimd.memset(VnA[:, :, 64:65], 1.0)
        nc.gpsimd.memset(VnB[:, :, 0:1], 1.0)

        def stage1_chunk(tci):
            """QKV^T for t-chunk tci: fills QT/KT[:, tci*TCH:...] and VnA/VnB."""
            xt = xt_pool.tile([128, 8, TCH], f32r, tag="xt")
            src = x_t[:, tci * TCH : (tci + 1) * TCH].rearrange(
                "(cc p) t -> p cc t", p=128
            )
            if tci == 0:
                # split the very first load so the first QKV matmul (which
                # only needs c-chunk 0) starts ~5us earlier
                for cc in range(8):
                    nc.sync.dma_start(xt[:, cc, :], src[:, cc, :])
            else:
                nc.sync.dma_start(xt[:], src)
            tsl = slice(tci * TCH, (tci + 1) * TCH)
            for jt in range(3):  # 0=Q, 1=K, 2=V
                ps = ps_work.tile([128, TCH], f32, tag="psw")
                for cc in range(8):
                    nc.tensor.matmul(
                        ps[:],
                        lhsT=wq_sb[:, cc, jt * 128 : (jt + 1) * 128],
                        rhs=xt[:, cc, :],
                        start=(cc == 0),
                        stop=(cc == 7),
                    )
                if jt < 2:
                    dest = QT if jt == 0 else KT
                    nc.vector.tensor_scalar(
                        dest[:, tsl], ps[:], bq_sb[:, jt : jt + 1], None, add
                    )
                else:
                    vs = vs_pool.tile([128, TCH], f32, tag="vs")
                    nc.vector.tensor_scalar(vs[:], ps[:], bq_sb[:, 2:3], None, add)
                    for q in range(TCH // 128):
                        tp = ps_work.tile([128, 128], f32, tag="psw")
                        nc.tensor.transpose(
                            tp[:], vs[:, q * 128 : (q + 1) * 128], ident[:]
                        )
                        gt = tci * (TCH // 128) + q  # global t-tile 0..63
                        nc.vector.tensor_copy(
                            out=VnA[:, gt, 0:64], in_=tp[:, 0:64]
                        )
                        nc.vector.tensor_copy(
                            out=VnB[:, gt, 64:128], in_=tp[:, 64:128]
                        )

        def attention_chunk(b, ic, yT):
            """S^T attention for i-chunk ic of batch b -> yT[:, ic*TCH:...]."""
            jt_n = 4 * ic + 4
            icol = b * T + ic * TCH
            psA = ps_y.tile([128, TCH], f32, tag="psy")  # yA 0:64, denA row 64
            psB = ps_y.tile([128, TCH], f32, tag="psy")  # denB row 0, yB 64:128
            for jt in range(jt_n):
                d = jt - 4 * ic
                col0 = 128 * d if d >= 0 else 0
                jcol = b * T + jt * 128
                s_ps = ps_big.tile([128, 1024], f32, tag="psb")
                # f32r matmuls with moving dim < 256 run at 1/4 rate; widen the
                # last diagonal block's QK to 256 (extra columns never read).
                qk0 = min(col0, 256)
                # QK^T row-packed: head A rows 0:64, head B rows 64:128
                nc.tensor.matmul(
                    s_ps[:, qk0:512],
                    lhsT=KT[0:64, jcol : jcol + 128],
                    rhs=QT[0:64, icol + qk0 : icol + 512],
                    start=True,
                    stop=True,
                )
                nc.tensor.matmul(
                    s_ps[:, 512 + qk0 : 1024],
                    lhsT=KT[64:128, jcol : jcol + 128],
                    rhs=QT[64:128, icol + qk0 : icol + 512],
                    start=True,
                    stop=True,
                )
                p = p_pool.tile([128, 2, TCH], bf16, tag="pp")
                s3 = s_ps[:].rearrange("p (h n) -> p h n", h=2)
                nc.scalar.activation(p[:, :, col0:], s3[:, :, col0:], Exp, scale=scale)
                if d >= 0:
                    # zero the upper triangle of the diagonal 128x128 block
                    nc.gpsimd.affine_select(
                        out=p[:, :, col0 : col0 + 128],
                        in_=p[:, :, col0 : col0 + 128],
                        pattern=[[0, 2], [1, 128]],
                        compare_op=is_ge,
                        fill=0.0,
                        base=0,
                        channel_multiplier=-1,
                    )
                gt = b * 16 + jt
                first, last = (jt == 0), (jt == jt_n - 1)
                nc.tensor.matmul(
                    psA[:, col0:512],
                    lhsT=VnA[:, gt, :],
                    rhs=p[:, 0, col0:],
                    start=first,
                    stop=last,
                    skip_group_check=True,
                )
                nc.tensor.matmul(
                    psB[:, col0:512],
                    lhsT=VnB[:, gt, :],
                    rhs=p[:, 1, col0:],
                    start=first,
                    stop=last,
                    skip_group_check=True,
                )
            # normalization: denA = psA[64], denB = psB[0]
            dn = rc_pool.tile([128, TCH], f32r, tag="dn")
            nc.vector.tensor_copy(out=dn[64:65, :], in_=psA[64:65, :])
            nc.vector.tensor_copy(out=dn[0:1, :], in_=psB[0:1, :])
            # K=1 broadcast matmuls: replicate each head's denominator row
            # across all 128 partitions (ISA requires dst partition base 0)
            bc1 = ps_work.tile([128, TCH], f32, tag="psw")
            bc2 = ps_work.tile([128, TCH], f32, tag="psw")
            nc.tensor.matmul(
                bc1[:], lhsT=ones1[64:65, :], rhs=dn[64:65, :], start=True, stop=True
            )
            nc.tensor.matmul(
                bc2[:], lhsT=ones1[0:1, :], rhs=dn[0:1, :], start=True, stop=True
            )
            # custom DVE ops misbehave at base_partition != 0 on HW: run each
            # reciprocal over the full 128 partitions and slice afterwards
            rc1 = rc_pool.tile([128, TCH], f32, tag="rc")
            rc2 = rc_pool.tile([128, TCH], f32, tag="rc")
            nc.vector.reciprocal_approx_fast(rc1[:], bc1[:])
            nc.vector.reciprocal_approx_fast(rc2[:], bc2[:])
            nc.vector.tensor_tensor(
                out=yT[0:64, ic * TCH : (ic + 1) * TCH],
                in0=psA[0:64, :],
                in1=rc1[0:64, :],
                op=mult,
            )
            nc.vector.tensor_tensor(
                out=yT[64:128, ic * TCH : (ic + 1) * TCH],
                in0=psB[64:128, :],
                in1=rc2[64:128, :],
                op=mult,
            )

        def proj_chunk(b, ic, yT):
            """out rows for i-chunk ic of batch b (4 row-tiles of 128)."""
            for q in range(TCH // 128):
                lt = ic * TCH + q * 128  # col in yT
                tt = (b * T + ic * TCH) // 128 + q  # global row-tile
                ob = ob_pool.tile([128, C], f32, tag="ob")
                for nh in range(2):
                    o_ps = ps_work.tile([128, 512], f32, tag="psw")
                    nc.tensor.matmul(
                        o_ps[:],
                        lhsT=yT[:, lt : lt + 128],
                        rhs=wp_sb[:, nh * 512 : (nh + 1) * 512],
                        start=True,
                        stop=True,
                    )
                    nc.vector.tensor_copy(
                        out=ob[:, nh * 512 : (nh + 1) * 512], in_=o_ps[:]
                    )
                nc.sync.dma_start(out[tt * 128 : (tt + 1) * 128, :], ob[:])

        for tci in range(4):
            stage1_chunk(tci)
        for b in range(B):
            yT = yt_pool.tile([128, T], f32r, tag="yt")
            for ic in range(T // TCH):
                attention_chunk(b, ic, yT)
                # prefetch next batch's QKV while this batch's attention runs
                if b + 1 < B:
                    stage1_chunk((b + 1) * 4 + ic)
                proj_chunk(b, ic, yT)

    nc.compile()
    return nc


def _get_nc():
    if "nc" not in _CACHE:
        _CACHE["nc"] = _build()
    return _CACHE["nc"]


def _run(inputs, trace=False):
    from concourse import bass_utils

    x = np.asarray(inputs["x"], dtype=np.float32)
    W_attn = np.asarray(inputs["W_attn"], dtype=np.float32)
    b_attn = np.asarray(inputs["b_attn"], dtype=np.float32)
    W_proj = np.asarray(inputs["W_proj"], dtype=np.float32)
    b_proj = np.asarray(inputs["b_proj"], dtype=np.float32)

    xT = np.ascontiguousarray(x.reshape(BT, C).T)  # [C, BT]
    ones_in = np.ones((128, 128), np.float32)

    in_maps = []
    for c in range(N_CORES):
        hA, hB = 2 * c, 2 * c + 1
        cols = []
        for part in range(3):  # q, k, v
            base = part * C
            cols.extend(range(base + hA * Dh, base + hA * Dh + Dh))
            cols.extend(range(base + hB * Dh, base + hB * Dh + Dh))
        cols = np.array(cols)
        in_maps.append(
            {
                "x_t": xT,
                "w_qkv": np.ascontiguousarray(W_attn[:, cols]),
                "b_qkv": np.ascontiguousarray(b_attn[cols].reshape(3, 128).T),
                "w_proj": np.ascontiguousarray(W_proj[c * 128 : (c + 1) * 128, :]),
                "ones_in": ones_in,
            }
        )

    nc = _get_nc()
    res = bass_utils.run_bass_kernel_spmd(
        nc, in_maps, core_ids=list(range(N_CORES)), trace=trace
    )
    acc = res.results[0]["out"].astype(np.float64)
    for c in range(1, N_CORES):
        acc += res.results[c]["out"]
    acc += b_proj
    return acc.reshape(B, T, C).astype(np.float32), res


def kernel(**inputs):
    out, _ = _run(inputs, trace=False)
    return out


def kernel_traced(**inputs):
    _, res = _run(inputs, trace=True)
    return res



# revision 13
# speedup vs baseline: 1.0859x; 1.0859x over previous
"""Causal self-attention (B=4, T=2048, C=1024, H=16) on 8 Trainium2 cores.

Sharding: tensor-parallel over heads. Core c owns heads (2c, 2c+1):
  - QKV projection in bf16: Q^T/K^T computed in [dim, token] layout with the
    host-pretransposed x^T (bf16) as the moving operand; V computed directly
    in [token, dim] layout (x^T t-block stationary, bf16 W_v moving, N=128)
    so no PE transposes are needed.
  - Attention in the S^T = [j, i] orientation so softmax'd probabilities
    feed the AV matmul untransposed. AV emission is software-pipelined one
    j-step behind QK so the PE always has independent work while the exp
    (ACT) of the previous step completes.
  - Softmax denominator rides the AV matmul (augmented-V ones column) and is
    broadcast across partitions with a single K=2 matmul of the two heads'
    reciprocal rows.
  - Output projection in f32r: per-core partial [BT, C]; partials summed on
    host (row-parallel tensor parallelism).
"""

import numpy as np

N_CORES = 8
B, T, C = 4, 2048, 1024
H, Dh = 16, 64
BT = B * T  # 8192
TCH = 512  # t-chunk (stage 1) and i-chunk (stage 2)

_CACHE = {}


def _build():
    import concourse.bacc as bacc
    import concourse.mybir as mybir
    import concourse.tile as tile
    from contextlib import ExitStack

    f32 = mybir.dt.float32
    f32r = mybir.dt.float32r
    bf16 = mybir.dt.bfloat16
    Exp = mybir.ActivationFunctionType.Exp
    add = mybir.AluOpType.add
    mult = mybir.AluOpType.mult
    is_ge = mybir.AluOpType.is_ge

    nc = bacc.Bacc(None, target_bir_lowering=False, debug=False)
    x_t = nc.dram_tensor("x_t", [C, BT], bf16, kind="ExternalInput").ap()
    w_qk = nc.dram_tensor("w_qk", [C, 256], bf16, kind="ExternalInput").ap()
    w_v = nc.dram_tensor("w_v", [C, 128], bf16, kind="ExternalInput").ap()
    b_qk = nc.dram_tensor("b_qk", [128, 2], f32, kind="ExternalInput").ap()
    b_v = nc.dram_tensor("b_v", [128, 128], f32, kind="ExternalInput").ap()
    w_proj = nc.dram_tensor("w_proj", [128, C], f32r, kind="ExternalInput").ap()
    out = nc.dram_tensor("out", [BT, C], f32, kind="ExternalOutput").ap()

    scale = 1.0 / float(np.sqrt(Dh))

    with tile.TileContext(nc) as tc, ExitStack() as ctx:
        persist = ctx.enter_context(tc.tile_pool(name="persist", bufs=1))
        xt_pool = ctx.enter_context(tc.tile_pool(name="xt", bufs=2))
        p_pool = ctx.enter_context(tc.tile_pool(name="pp", bufs=8))
        rc_pool = ctx.enter_context(tc.tile_pool(name="rc", bufs=3))
        yt_pool = ctx.enter_context(tc.tile_pool(name="yt", bufs=2))
        ob_pool = ctx.enter_context(tc.tile_pool(name="ob", bufs=3))
        # PSUM 8 banks: s_ps 2x[128,1024]=4, psA/psB 2x[128,512]=2, work 2
        ps_big = ctx.enter_context(tc.tile_pool(name="psb", bufs=2, space="PSUM"))
        ps_y = ctx.enter_context(tc.tile_pool(name="psy", bufs=2, space="PSUM"))
        ps_work = ctx.enter_context(tc.tile_pool(name="psw", bufs=2, space="PSUM"))

        QT = persist.tile([128, BT], bf16, tag="QT")
        KT = persist.tile([128, BT], bf16, tag="KT")
        # augmented V, bf16: per global t-tile gt a [128, 128] stationary block
        #   VnA[:, gt, 0:64] = V_A, [.., 64] = 1.0, rest 0
        #   VnB[:, gt, 0] = 1.0, [.., 64:128] = V_B, rest 0
        VnA = persist.tile([128, 64, 128], bf16, tag="VnA")
        VnB = persist.tile([128, 64, 128], bf16, tag="VnB")
        wqk_sb = persist.tile([128, 8, 256], bf16, tag="wqk")
        wv_sb = persist.tile([128, 8, 128], bf16, tag="wv")
        wp_sb = persist.tile([128, C], f32r, tag="wp")
        bqk_sb = persist.tile([128, 2], f32, tag="bqk")
        bv_sb = persist.tile([128, 128], f32, tag="bv")

        # weights via SWDGE ring (Pool queue) so they run ahead of x chunks
        # on the shared DMA device; Q/K weights first (first-needed).
        nc.gpsimd.dma_start(wqk_sb[:], w_qk.rearrange("(cc p) j -> p cc j", p=128))
        nc.gpsimd.dma_start(wv_sb[:], w_v.rearrange("(cc p) j -> p cc j", p=128))
        nc.gpsimd.dma_start(bqk_sb[:], b_qk[:])
        nc.gpsimd.dma_start(bv_sb[:], b_v[:])
        nc.gpsimd.memset(VnA[:], 0.0)
        nc.gpsimd.memset(VnB[:], 0.0)
        nc.gpsimd.memset(VnA[:, :, 64:65], 1.0)
        nc.gpsimd.memset(VnB[:, :, 0:1], 1.0)
        nc.gpsimd.dma_start(wp_sb[:], w_proj[:])

        def stage1_chunk(tci):
            """QKV for t-chunk tci: fills QT/KT[:, tci*TCH:...] and VnA/VnB."""
            xt = xt_pool.tile([128, 8, TCH], bf16, tag="xt")
            src = x_t[:, tci * TCH : (tci + 1) * TCH].rearrange(
                "(cc p) t -> p cc t", p=128
            )
            if tci == 0:
                # split the very first load so the first QK matmul (which
                # only needs c-chunk 0) starts earlier
                for cc in range(8):
                    nc.sync.dma_start(xt[:, cc, :], src[:, cc, :])
            else:
                nc.sync.dma_start(xt[:], src)
            tsl = slice(tci * TCH, (tci + 1) * TCH)
            for jt in range(2):  # 0=Q, 1=K
                ps = ps_work.tile([128, TCH], f32, tag="psw")
                for cc in range(8):
                    nc.tensor.matmul(
                        ps[:],
                        lhsT=wqk_sb[:, cc, jt * 128 : (jt + 1) * 128],
                        rhs=xt[:, cc, :],
                        start=(cc == 0),
                        stop=(cc == 7),
                    )
                dest = QT if jt == 0 else KT
                nc.vector.tensor_scalar(
                    dest[:, tsl], ps[:], bqk_sb[:, jt : jt + 1], None, add
                )
            # V direct in [token, dim] layout: x^T t-block stationary,
            # W_v moving (bf16, N=128); 4 t-tiles side by side in one bank.
            vp = ps_work.tile([128, TCH], f32, tag="psw")
            for q in range(4):
                for cc in range(8):
                    nc.tensor.matmul(
                        vp[:, q * 128 : (q + 1) * 128],
                        lhsT=xt[:, cc, q * 128 : (q + 1) * 128],
                        rhs=wv_sb[:, cc, :],
                        start=(cc == 0),
                        stop=(cc == 7),
                        skip_group_check=True,
                    )
            for q in range(4):
                gt = tci * 4 + q
                nc.vector.tensor_tensor(
                    out=VnA[:, gt, 0:64],
                    in0=vp[:, q * 128 : q * 128 + 64],
                    in1=bv_sb[:, 0:64],
                    op=add,
                )
                nc.vector.tensor_tensor(
                    out=VnB[:, gt, 64:128],
                    in0=vp[:, q * 128 + 64 : (q + 1) * 128],
                    in1=bv_sb[:, 64:128],
                    op=add,
                )

        def attention_chunk(b, ic, yT):
            """S^T attention for i-chunk ic of batch b -> yT[:, ic*TCH:...]."""
            jt_n = 4 * ic + 4
            icol = b * T + ic * TCH
            psA = ps_y.tile([128, TCH], f32, tag="psy")  # yA 0:64, denA row 64
            psB = ps_y.tile([128, TCH], f32, tag="psy")  # denB row 0, yB 64:128
            p_tiles = [None] * jt_n
            col0s = [None] * jt_n

            def emit_av(jt):
                p, col0 = p_tiles[jt], col0s[jt]
                gt = b * 16 + jt
                first, last = (jt == 0), (jt == jt_n - 1)
                nc.tensor.matmul(
                    psA[:, col0:512],
                    lhsT=VnA[:, gt, :],
                    rhs=p[:, 0, col0:],
                    start=first,
                    stop=last,
                    skip_group_check=True,
                )
                nc.tensor.matmul(
                    psB[:, col0:512],
                    lhsT=VnB[:, gt, :],
                    rhs=p[:, 1, col0:],
                    start=first,
                    stop=last,
                    skip_group_check=True,
                )

            for jt in range(jt_n):
                d = jt - 4 * ic
                col0 = 128 * d if d >= 0 else 0
                col0s[jt] = col0
                jcol = b * T + jt * 128
                s_ps = ps_big.tile([128, 1024], f32, tag="psb")
                # QK^T row-packed: head A rows 0:64, head B rows 64:128
                nc.tensor.matmul(
                    s_ps[:, col0:512],
                    lhsT=KT[0:64, jcol : jcol + 128],
                    rhs=QT[0:64, icol + col0 : icol + 512],
                    start=True,
                    stop=True,
                )
                nc.tensor.matmul(
                    s_ps[:, 512 + col0 : 1024],
                    lhsT=KT[64:128, jcol : jcol + 128],
                    rhs=QT[64:128, icol + col0 : icol + 512],
                    start=True,
                    stop=True,
                )
                p = p_pool.tile([128, 2, TCH], bf16, tag="pp")
                p_tiles[jt] = p
                s3 = s_ps[:].rearrange("p (h n) -> p h n", h=2)
                nc.scalar.activation(p[:, :, col0:], s3[:, :, col0:], Exp, scale=scale)
                if d >= 0:
                    # zero the upper triangle of the diagonal 128x128 block
                    nc.gpsimd.affine_select(
                        out=p[:, :, col0 : col0 + 128],
                        in_=p[:, :, col0 : col0 + 128],
                        pattern=[[0, 2], [1, 128]],
                        compare_op=is_ge,
                        fill=0.0,
                        base=0,
                        channel_multiplier=-1,
                    )
                # software pipeline: AV one j-step behind QK/exp
                if jt >= 1:
                    emit_av(jt - 1)
            emit_av(jt_n - 1)

            # normalization: denA = psA[64], denB = psB[0]; reciprocal rows,
            # then one K=2 matmul broadcasts rcA to partitions 0:64 and rcB
            # to 64:128.
            # denA lives at psA partition 64 — stage it to partition 0 first
            # (custom DVE ops and partition_broadcast need base-partition 0).
            dn = rc_pool.tile([128, 2 * TCH], f32, tag="dn")
            nc.vector.tensor_copy(out=dn[0:1, 0:TCH], in_=psA[64:65, :])
            nc.vector.reciprocal_approx_fast(dn[0:1, TCH : 2 * TCH], psB[0:1, :])
            nc.vector.reciprocal_approx_fast(dn[0:1, 0:TCH], dn[0:1, 0:TCH])
            rcA = rc_pool.tile([128, TCH], f32, tag="rcA")
            rcB = rc_pool.tile([128, TCH], f32, tag="rcB")
            nc.gpsimd.partition_broadcast(rcA[:, :], dn[0:1, 0:TCH])
            nc.gpsimd.partition_broadcast(rcB[:, :], dn[0:1, TCH : 2 * TCH])
            nc.vector.tensor_tensor(
                out=yT[0:64, ic * TCH : (ic + 1) * TCH],
                in0=psA[0:64, :],
                in1=rcA[0:64, :],
                op=mult,
            )
            nc.vector.tensor_tensor(
                out=yT[64:128, ic * TCH : (ic + 1) * TCH],
                in0=psB[64:128, :],
                in1=rcB[64:128, :],
                op=mult,
            )

        def proj_chunk(b, ic, yT):
            """out rows for i-chunk ic of batch b (4 row-tiles of 128)."""
            for q in range(TCH // 128):
                lt = ic * TCH + q * 128  # col in yT
                tt = (b * T + ic * TCH) // 128 + q  # global row-tile
                ob = ob_pool.tile([128, C], f32, tag="ob")
                for nh in range(2):
                    o_ps = ps_work.tile([128, 512], f32, tag="psw")
                    nc.tensor.matmul(
                        o_ps[:],
                        lhsT=yT[:, lt : lt + 128],
                        rhs=wp_sb[:, nh * 512 : (nh + 1) * 512],
                        start=True,
                        stop=True,
                    )
                    nc.vector.tensor_copy(
                        out=ob[:, nh * 512 : (nh + 1) * 512], in_=o_ps[:]
                    )
                nc.sync.dma_start(out[tt * 128 : (tt + 1) * 128, :], ob[:])

        stage1_chunk(0)
        yT = None
        for step in range(16):
            b, ic = divmod(step, 4)
            if ic == 0:
                yT = yt_pool.tile([128, T], f32r, tag="yt")
            attention_chunk(b, ic, yT)
            if step + 1 < 16:
                stage1_chunk(step + 1)
            proj_chunk(b, ic, yT)

    nc.compile()
    return nc


def _get_nc():
    if "nc" not in _CACHE:
        _CACHE["nc"] = _build()
    return _CACHE["nc"]


def _run(inputs, trace=False):
    import ml_dtypes
    from concourse import bass_utils

    bfloat16 = ml_dtypes.bfloat16

    x = np.asarray(inputs["x"], dtype=np.float32)
    W_attn = np.asarray(inputs["W_attn"], dtype=np.float32)
    b_attn = np.asarray(inputs["b_attn"], dtype=np.float32)
    W_proj = np.asarray(inputs["W_proj"], dtype=np.float32)
    b_proj = np.asarray(inputs["b_proj"], dtype=np.float32)

    xT = np.ascontiguousarray(x.reshape(BT, C).T).astype(bfloat16)  # [C, BT]

    in_maps = []
    for c in range(N_CORES):
        hA, hB = 2 * c, 2 * c + 1
        qk_cols = []
        for part in range(2):  # q, k
            base = part * C
            qk_cols.extend(range(base + hA * Dh, base + hA * Dh + Dh))
            qk_cols.extend(range(base + hB * Dh, base + hB * Dh + Dh))
        qk_cols = np.array(qk_cols)
        v_cols = np.array(
            list(range(2 * C + hA * Dh, 2 * C + hA * Dh + Dh))
            + list(range(2 * C + hB * Dh, 2 * C + hB * Dh + Dh))
        )
        in_maps.append(
            {
                "x_t": xT,
                "w_qk": np.ascontiguousarray(W_attn[:, qk_cols]).astype(bfloat16),
                "w_v": np.ascontiguousarray(W_attn[:, v_cols]).astype(bfloat16),
                "b_qk": np.ascontiguousarray(b_attn[qk_cols].reshape(2, 128).T),
                "b_v": np.ascontiguousarray(
                    np.broadcast_to(b_attn[v_cols], (128, 128))
                ),
                "w_proj": np.ascontiguousarray(W_proj[c * 128 : (c + 1) * 128, :]),
            }
        )

    nc = _get_nc()
    res = bass_utils.run_bass_kernel_spmd(
        nc, in_maps, core_ids=list(range(N_CORES)), trace=trace
    )
    acc = res.results[0]["out"].astype(np.float64)
    for c in range(1, N_CORES):
        acc += res.results[c]["out"]
    acc += b_proj
    return acc.reshape(B, T, C).astype(np.float32), res


def kernel(**inputs):
    out, _ = _run(inputs, trace=False)
    return out


def kernel_traced(**inputs):
    _, res = _run(inputs, trace=True)
    return res
